# revision 18
# baseline (speedup 1.0000x reference)
"""MoE expert-gating kernel for 8 Trainium2 NeuronCores.

Problem (nn_ExpertGating): router MLP (H->H relu, H->E) + softmax + top-2
gating + weighted combine of per-expert outputs.

Sharding: data-parallel over the B*S=8192 tokens -> 1024 tokens per core.
Each core runs the full router for its tokens and combines its slice of all
8 experts' outputs.  No collectives needed; host concatenates the slices.

Per-core pipeline (T=1024 tokens, H=1024, E=8):
  1. x arrives host-pre-transposed, pre-split into fp16 hi + bf16 lo
     halves, AND pre-shuffled into the exact SBUF layout per 256-token
     segment, so every input DMA moves >=4KB-contiguous partition lines
     (512B-run rearranges measured ~60GB/s vs ~250GB/s for 4KB runs).
  2. hT = relu(W1.T @ xT + b1) via 3 fp16/bf16 matmul passes (hi*hi;
     hi*lo + lo*hi), 1 cycle/row each.  This is the PE floor: the PE
     multiplies FP22 (~12-13 bit) operands, and both x and W1 need ~19
     bits for the logits to rank top-2 correctly (min top-2/3 margin on
     this data is ~5e-6; fp16x3 keeps logit error ~1e-6).  fp32 matmul is
     4 cycles/row (2 HW instructions) and f32r truncates both sides to
     ~12 bits (probed on HW), so neither beats 3x fp16.
  3. logitsT[e, t] += W2.T @ hT accumulated per m-block right after each
     relu (fp32: W2 also needs >13 bits), so segment logits are ready
     immediately after the segment's last stage-2 matmul.
  4. transpose logit chunks to [t, E] via PE (8x8 identity), softmax,
     then a dense top-2 gate row per token: gd[t,e] = probs masked by
     (exps >= 2nd-max) -- exactly 2 survivors per token (no exact ties
     on fp32 data; logit margins >= 5e-6 >> fp32 ulp).
  5. DENSE combine: out[t] = sum_e gd[t,e] * eo[e,t].  All 8 experts'
     rows stream in as fp16 via the fast HWDGE rings (host-permuted to
     [chunk, p, e, h] so partition lines are 16KB contiguous), with NO
     dependency on the router -- unlike the previous indirect-gather
     design, whose single SWDGE queue cost ~6us per 128-row gather,
     could only start after the first segment's logits, and dominated
     the tail.  The 8 multiply-accumulates per chunk split h-halves
     across the DVE and the (otherwise idle) GpSimd engine.

DMA notes: every dma_start costs ~610ns of dispatch on the issuing queue;
inputs are batched into few transfers split across BOTH HWDGE rings
(SP: constants + xT + even dense chunks; Activation: W1 k-quarters + odd
dense chunks), each a separate tile so tile-level dependencies stay
fine-grained.  Dense chunks 4..7 are dispatched as chunks 0..3 retire so
a tile-slot wait never stalls the ring.  Segment 0 runs its k-loop
outermost, consuming W1 k-quarters as they land.
"""

import numpy as np

B, S, H, E = 4, 2048, 1024, 8
N_CORES = 8
T = (B * S) // N_CORES  # tokens per core
P = 128  # partitions
TCH = T // P  # token chunks per core (8)
KT = H // P  # contraction tiles (8)
HAL = 512  # psum pad width (full bank)
SEGS = [(0, 2), (2, 4), (4, 6), (6, 8)]
NSEG = len(SEGS)
SW = 2 * P  # segment width (tokens)
NQ = KT // 2  # w1 delivered in k-quarters
NEOB = 4  # dense expert-chunk tiles in flight
CBLOB = KT + KT * E + E + 1  # b1 | w2 | ident | b2  = 81 fp32 columns

_compiled_nc = None


def _build():
    import concourse.bacc as bacc
    import concourse.tile as tile
    from concourse import mybir

    f32 = mybir.dt.float32
    f16 = mybir.dt.float16
    bf16 = mybir.dt.bfloat16
    nc = bacc.Bacc("TRN2", target_bir_lowering=False, debug=False,
                   num_devices=N_CORES)

    # all inputs are host-shuffled to exact SBUF layout (>=4KB lines)
    xh = nc.dram_tensor("xh", [NSEG, P, KT, SW], f16, kind="ExternalInput").ap()
    xl = nc.dram_tensor("xl", [NSEG, P, KT, SW], bf16, kind="ExternalInput").ap()
    eod = nc.dram_tensor("eod", [TCH, P, E, H], f16, kind="ExternalInput").ap()
    w1h = nc.dram_tensor("w1h", [NQ, P, 2, H], f16, kind="ExternalInput").ap()
    w1l = nc.dram_tensor("w1l", [NQ, P, 2, H], bf16, kind="ExternalInput").ap()
    cblob = nc.dram_tensor("cblob", [P, CBLOB], f32, kind="ExternalInput").ap()
    out = nc.dram_tensor("out", [T, H], f32, kind="ExternalOutput").ap()

    with tile.TileContext(nc) as tc:
        with (
            tc.tile_pool(name="singles", bufs=1) as singles,
            tc.tile_pool(name="eodpool", bufs=NEOB) as eodpool,
            tc.tile_pool(name="accpool", bufs=3) as accpool,
            tc.tile_pool(name="smalls", bufs=8) as smalls,
            tc.tile_pool(name="ltpool", bufs=2) as ltpool,
            tc.tile_pool(name="psum", bufs=8, space="PSUM") as psum,
        ):
            # ---- input DMAs: SP ring = constants + xT, Activation ring =
            # W1 k-quarters (separate tiles per transfer -> fine deps) ----
            cb = singles.tile([P, CBLOB], f32)
            nc.sync.dma_start(out=cb[:], in_=cblob)
            b1_sb = cb[:, 0:KT]                       # b1_sb[p,m] = b1[m*128+p]
            w2col = lambda j: cb[:, KT + j * E:KT + (j + 1) * E]  # W2[j*128+p, e]
            ident = cb[0:E, KT + KT * E:KT + KT * E + E]          # eye(8)
            b2_sb = cb[0:E, CBLOB - 1:CBLOB]

            xh_s, xl_s, w1h_q, w1l_q = [], [], [], []
            for s in range(2):  # seg0, seg1 x before w1 (k-outer needs x first)
                th = singles.tile([P, KT, SW], f16, tag="xhs", name=f"xhs{s}",
                                  bufs=NSEG)
                nc.sync.dma_start(out=th[:], in_=xh[s])
                tl = singles.tile([P, KT, SW], bf16, tag="xls", name=f"xls{s}",
                                  bufs=NSEG)
                nc.sync.dma_start(out=tl[:], in_=xl[s])
                xh_s.append(th)
                xl_s.append(tl)
            for q in range(NQ):
                th = singles.tile([P, 2, H], f16, tag="w1hq", name=f"w1hq{q}",
                                  bufs=NQ)
                nc.scalar.dma_start(out=th[:], in_=w1h[q])
                tl = singles.tile([P, 2, H], bf16, tag="w1lq", name=f"w1lq{q}",
                                  bufs=NQ)
                nc.scalar.dma_start(out=tl[:], in_=w1l[q])
                w1h_q.append(th)
                w1l_q.append(tl)
            for s in range(2, NSEG):
                th = singles.tile([P, KT, SW], f16, tag="xhs", name=f"xhs{s}",
                                  bufs=NSEG)
                nc.sync.dma_start(out=th[:], in_=xh[s])
                tl = singles.tile([P, KT, SW], bf16, tag="xls", name=f"xls{s}",
                                  bufs=NSEG)
                nc.sync.dma_start(out=tl[:], in_=xl[s])
                xh_s.append(th)
                xl_s.append(tl)

            eo_t = {}

            def load_dense(c):
                t = eodpool.tile([P, E, H], f16, tag="eod", name=f"eod{c}")
                ring = nc.sync if c % 2 == 0 else nc.scalar
                ring.dma_start(out=t[:], in_=eod[c])
                eo_t[c] = t

            for c in range(NEOB):  # first 4 dense chunks up front
                load_dense(c)

            hT = singles.tile([P, KT, T], f32)  # hT[p,m,t] = relu(x@W1+b1)[t, m*128+p]

            def mm3(ps, si, m, k, start, stop):
                msl = slice(m * P, (m + 1) * P)
                wh = w1h_q[k // 2][:, k % 2, msl]
                wl = w1l_q[k // 2][:, k % 2, msl]
                nc.tensor.matmul(ps[:], lhsT=wh, rhs=xh_s[si][:, k, :],
                                 start=start, stop=False)
                nc.tensor.matmul(ps[:], lhsT=wl, rhs=xh_s[si][:, k, :],
                                 start=False, stop=False)
                nc.tensor.matmul(ps[:], lhsT=wh, rhs=xl_s[si][:, k, :],
                                 start=False, stop=stop)

            def relu_stage3(ps, ps3, m, sl):
                nc.scalar.activation(
                    out=hT[:, m, sl], in_=ps[:],
                    func=mybir.ActivationFunctionType.Relu,
                    bias=b1_sb[:, m:m + 1], scale=1.0)
                nc.tensor.matmul(
                    ps3[:], lhsT=w2col(m), rhs=hT[:, m, sl],
                    start=(m == 0), stop=(m == KT - 1))

            # phase A: softmax + dense top-2 gate row for one 128-token chunk
            def chunk_phase_a(lT, c0, tch):
                a = tch - c0
                pl = psum.tile([P, E], f32, tag="ps", name="pl",
                               padded_shape=[P, HAL])
                nc.tensor.transpose(pl[:], lT[:, a * P:(a + 1) * P], ident)
                negmax = smalls.tile([P, 1], f32, tag="negmax", name="negmax")
                nc.vector.reduce_max(negmax[:], pl[:],
                                     axis=mybir.AxisListType.X, negate=True)
                exps = smalls.tile([P, E], f32, tag="exps", name="exps")
                nc.scalar.activation(exps[:], pl[:],
                                     func=mybir.ActivationFunctionType.Exp,
                                     bias=negmax[:], scale=1.0)
                ssum = smalls.tile([P, 1], f32, tag="ssum", name="ssum")
                nc.vector.reduce_sum(ssum[:], exps[:], axis=mybir.AxisListType.X)
                rs = smalls.tile([P, 1], f32, tag="rs", name="rs")
                nc.vector.reciprocal(rs[:], ssum[:])
                mx8 = smalls.tile([P, 8], f32, tag="mx8", name="mx8")
                nc.vector.max(mx8[:], exps[:])
                # gd[t,e] = (exps >= 2nd max) * exps * (1/sum): the top-2
                # gates in dense form (exactly 2 nonzero -- no fp32 ties).
                # Per-partition scalars ride scalar_tensor_tensor (proven
                # path); TensorScalarPtr+is_ge dies in walrus codegen.
                diff = smalls.tile([P, E], f32, tag="mask", name="diff")
                nc.vector.scalar_tensor_tensor(
                    out=diff[:], in0=exps[:], scalar=mx8[:, 1:2], in1=exps[:],
                    op0=mybir.AluOpType.subtract, op1=mybir.AluOpType.bypass)
                mask = smalls.tile([P, E], f32, tag="mask2", name="mask")
                nc.vector.tensor_scalar(mask[:], diff[:], 0.0, None,
                                        op0=mybir.AluOpType.is_ge)
                gd = smalls.tile([P, E], f32, tag="gd", name="gd")
                nc.vector.scalar_tensor_tensor(
                    out=gd[:], in0=exps[:], scalar=rs[:], in1=mask[:],
                    op0=mybir.AluOpType.mult, op1=mybir.AluOpType.mult)
                return (tch, gd)

            # phase B: dense weighted combine on the DVE, all-fp16 for the
            # 2x 16-bit datapath (fp16 interim rounding ~5e-4 rel << 2e-2
            # budget), then an fp16->fp32 convert on the idle scalar engine
            def chunk_phase_b(st):
                tch, gd = st
                eo_c = eo_t[tch]
                acc = accpool.tile([P, H], f16, tag="acc", name="acc")
                nc.vector.tensor_scalar_mul(acc[:], eo_c[:, 0, :], gd[:, 0:1])
                for e in range(1, E):
                    nc.vector.scalar_tensor_tensor(
                        out=acc[:], in0=eo_c[:, e, :],
                        scalar=gd[:, e:e + 1], in1=acc[:],
                        op0=mybir.AluOpType.mult, op1=mybir.AluOpType.add)
                acc32 = accpool.tile([P, H], f32, tag="acc32", name="acc32")
                nc.scalar.copy(out=acc32[:], in_=acc[:])
                nc.sync.dma_start(out=out[tch * P:(tch + 1) * P, :], in_=acc32[:])
                # refill the dense pipeline as each chunk retires
                nxt = tch + NEOB if tch < TCH - NEOB else None
                if tch + NEOB < TCH:
                    load_dense(tch + NEOB)

            for si, (c0, c1) in enumerate(SEGS):
                sl = slice(c0 * P, c1 * P)
                W = (c1 - c0) * P
                # ---- stage 2 (+ interleaved stage 3) ----
                if si == 0:
                    # k-outer: consume W1/xT k-blocks as the DMA delivers
                    # them.  ps3 must be allocated AFTER the 8 stage-2
                    # accumulators: the pool has exactly 8 slots and a 9th
                    # live tile ahead of them deadlocks the slot ring.
                    ps_m = [psum.tile([P, W], f32, tag="ps", name=f"ps{m}",
                                      padded_shape=[P, HAL]) for m in range(KT)]
                    for k in range(KT):
                        for m in range(KT):
                            mm3(ps_m[m], si, m, k, k == 0, k == KT - 1)
                    ps3 = psum.tile([E, W], f32, tag="ps", name="ps3",
                                    padded_shape=[E, HAL])
                    for m in range(KT):
                        relu_stage3(ps_m[m], ps3, m, sl)
                else:
                    ps3 = psum.tile([E, W], f32, tag="ps", name="ps3",
                                    padded_shape=[E, HAL])
                    for m in range(KT):
                        ps = psum.tile([P, W], f32, tag="ps", name="ps",
                                       padded_shape=[P, HAL])
                        for k in range(KT):
                            mm3(ps, si, m, k, k == 0, k == KT - 1)
                        relu_stage3(ps, ps3, m, sl)

                lT = ltpool.tile([E, W], f32, tag="lT", name="lT",
                                 padded_shape=[E, HAL])
                nc.scalar.activation(out=lT[:], in_=ps3[:],
                                     func=mybir.ActivationFunctionType.Identity,
                                     bias=b2_sb, scale=1.0)
                # last segment: final chunk first so the tail's combine
                # chain starts on the critical chunk
                order = range(c0, c1) if si < NSEG - 1 else reversed(range(c0, c1))
                for tch in order:
                    chunk_phase_b(chunk_phase_a(lT, c0, tch))

    nc.compile()
    return nc


def _get_nc():
    global _compiled_nc
    if _compiled_nc is None:
        _compiled_nc = _build()
    return _compiled_nc


def _split_hi_lo(a):
    """fp16 hi + bf16 lo split of an fp32 array (lo unscaled; bf16's
    exponent range covers it)."""
    import ml_dtypes
    a = np.asarray(a, dtype=np.float32)
    hi = a.astype(np.float16)
    lo = (a.astype(np.float64) - hi.astype(np.float64)).astype(ml_dtypes.bfloat16)
    return hi, lo


def make_in_maps(hidden_states, expert_outputs, W1, b1, W2, b2):
    hs = np.ascontiguousarray(np.asarray(hidden_states, dtype=np.float32)).reshape(B * S, H)
    eo = np.asarray(expert_outputs, dtype=np.float32).reshape(E, B * S, H)
    w1hi, w1lo = _split_hi_lo(W1)
    # w1 k-quarter blobs in SBUF layout: [q, p, kk, m] = W1[(2q+kk)*128+p, m]
    w1hq = np.ascontiguousarray(
        w1hi.reshape(NQ, 2, P, H).transpose(0, 2, 1, 3))
    w1lq = np.ascontiguousarray(
        w1lo.reshape(NQ, 2, P, H).transpose(0, 2, 1, 3))
    b1v = np.asarray(b1, dtype=np.float32)
    w2 = np.asarray(W2, dtype=np.float32)
    b2v = np.asarray(b2, dtype=np.float32)
    # constants blob: b1 | w2 | ident | b2, all in on-chip layout
    cb = np.zeros((P, CBLOB), dtype=np.float32)
    cb[:, 0:KT] = b1v.reshape(KT, P).T                    # b1[m*128+p]
    cb[:, KT:KT + KT * E] = w2.reshape(KT, P, E).transpose(1, 0, 2).reshape(P, KT * E)
    cb[0:E, KT + KT * E:KT + KT * E + E] = np.eye(E, dtype=np.float32)
    cb[0:E, CBLOB - 1] = b2v
    in_maps = []
    for c in range(N_CORES):
        sl = slice(c * T, (c + 1) * T)
        xhi, xlo = _split_hi_lo(hs[sl].T)  # [H, T]
        # per-seg SBUF-layout blobs: [s, p, k, u] = xT[k*128+p, s*SW+u]
        xhb = np.ascontiguousarray(
            xhi.reshape(KT, P, NSEG, SW).transpose(2, 1, 0, 3))
        xlb = np.ascontiguousarray(
            xlo.reshape(KT, P, NSEG, SW).transpose(2, 1, 0, 3))
        # dense expert rows per chunk: [tch, p, e, h] = eo[e, tch*128+p, h]
        eodb = np.ascontiguousarray(
            eo[:, sl, :].reshape(E, TCH, P, H).transpose(1, 2, 0, 3)
            .astype(np.float16))
        in_maps.append({
            "xh": xhb, "xl": xlb, "eod": eodb,
            "w1h": w1hq, "w1l": w1lq, "cblob": cb,
        })
    return in_maps


def kernel(hidden_states, expert_outputs, W1, b1, W2, b2, k=2):
    from concourse.bass_utils import run_bass_kernel_spmd

    in_maps = make_in_maps(hidden_states, expert_outputs, W1, b1, W2, b2)
    nc = _get_nc()
    res = run_bass_kernel_spmd(nc, in_maps, core_ids=list(range(N_CORES)))
    full = np.concatenate([res.results[c]["out"] for c in range(N_CORES)], axis=0)
    return full.reshape(B, S, H)


# revision 19
# speedup vs baseline: 1.0139x; 1.0139x over previous
"""MoE expert-gating kernel for 8 Trainium2 NeuronCores.

Problem (nn_ExpertGating): router MLP (H->H relu, H->E) + softmax + top-2
gating + weighted combine of per-expert outputs.

Sharding: data-parallel over the B*S=8192 tokens -> 1024 tokens per core.
Each core runs the full router for its tokens and combines its slice of all
8 experts' outputs.  No collectives needed; host concatenates the slices.

Per-core pipeline (T=1024 tokens, H=1024, E=8):
  1. x arrives host-pre-transposed, pre-split into fp16 hi + bf16 lo
     halves, AND pre-shuffled into the exact SBUF layout per 256-token
     segment, so every input DMA moves 4KB-contiguous partition lines
     (512B-run rearranges measured ~60GB/s vs ~250GB/s for 4KB runs).
  2. hT = relu(W1.T @ xT + b1) via 3 fp16/bf16 matmul passes (hi*hi;
     hi*lo + lo*hi), 1 cycle/row each.  This is the PE floor: the PE
     multiplies FP22 (~12-13 bit) operands, and both x and W1 need ~19
     bits for the logits to rank top-2 correctly (min top-2/3 margin on
     this data is ~5e-6; fp16x3 keeps logit error ~1e-6).  fp32 matmul is
     4 cycles/row (2 HW instructions) and f32r truncates both sides to
     ~12 bits (probed on HW), so neither beats 3x fp16.
  3. logitsT[e, t] += W2.T @ hT accumulated per m-block right after each
     relu (fp32: W2 also needs >13 bits), so segment logits are ready
     immediately after the segment's last stage-2 matmul.
  4. transpose logit chunks to [t, E] via PE (8x8 identity), softmax,
     top-2 via max8 + max_index, and launch the indirect-DMA gathers of
     each token's 2 selected expert rows.  expert_outputs are host-cast
     to fp16 (4 MB gathered instead of 8; rounding error ~2e-4 rel vs
     the 2e-2 budget): the SWDGE gather queue runs at only ~90-180GB/s
     and is the tail bottleneck.
  5. the weighted combine (out = g0*row0 + g1*row1) for segment s's
     chunks is DEFERRED TWO segments (emitted mid segment s+2's m-loop)
     and runs on the DVE: engine queues are strictly in-order, and a
     combine emitted earlier waits on its gather transfer,
     head-of-line-blocking the relu chain and stalling the PE.

DMA notes: every dma_start costs ~610ns of dispatch on the issuing queue,
so inputs are batched into a few transfers split across BOTH HWDGE rings
(SP: constants + xT segments; Activation: W1 k-quarters), each a separate
tile so tile-level dependencies stay fine-grained.  Segment 0 runs its
k-loop outermost, consuming W1 k-quarters as they land.
"""

import numpy as np

B, S, H, E = 4, 2048, 1024, 8
N_CORES = 8
T = (B * S) // N_CORES  # tokens per core
P = 128  # partitions
TCH = T // P  # token chunks per core (8)
KT = H // P  # contraction tiles (8)
HAL = 512  # psum pad width (full bank)
SEGS = [(0, 2), (2, 4), (4, 6), (6, 8)]
NSEG = len(SEGS)
SW = 2 * P  # segment width (tokens)
NQ = KT // 2  # w1 delivered in k-quarters
CBLOB = KT + KT * E + E + 1  # b1 | w2 | ident | b2  = 81 fp32 columns

_compiled_nc = None


def _build():
    import concourse.bacc as bacc
    import concourse.bass as bass
    import concourse.tile as tile
    from concourse import mybir

    f32 = mybir.dt.float32
    f16 = mybir.dt.float16
    bf16 = mybir.dt.bfloat16
    u32 = mybir.dt.uint32
    nc = bacc.Bacc("TRN2", target_bir_lowering=False, debug=False,
                   num_devices=N_CORES)

    # all inputs are host-shuffled to exact SBUF layout (4KB partition lines)
    xh = nc.dram_tensor("xh", [NSEG, P, KT, SW], f16, kind="ExternalInput").ap()
    xl = nc.dram_tensor("xl", [NSEG, P, KT, SW], bf16, kind="ExternalInput").ap()
    eo = nc.dram_tensor("eo", [E * T, H], f16, kind="ExternalInput").ap()
    w1h = nc.dram_tensor("w1h", [NQ, P, 2, H], f16, kind="ExternalInput").ap()
    w1l = nc.dram_tensor("w1l", [NQ, P, 2, H], bf16, kind="ExternalInput").ap()
    cblob = nc.dram_tensor("cblob", [P, CBLOB], f32, kind="ExternalInput").ap()
    iotad = nc.dram_tensor("iota", [P, 1], u32, kind="ExternalInput").ap()
    out = nc.dram_tensor("out", [T, H], f32, kind="ExternalOutput").ap()

    with tile.TileContext(nc) as tc:
        with (
            tc.tile_pool(name="singles", bufs=1) as singles,
            tc.tile_pool(name="eopool", bufs=6) as eopool,
            tc.tile_pool(name="accpool", bufs=3) as accpool,
            tc.tile_pool(name="smalls", bufs=8) as smalls,
            tc.tile_pool(name="ltpool", bufs=2) as ltpool,
            tc.tile_pool(name="psum", bufs=8, space="PSUM") as psum,
        ):
            # ---- input DMAs: SP ring = constants + xT, Activation ring =
            # W1 k-quarters (separate tiles per transfer -> fine deps) ----
            cb = singles.tile([P, CBLOB], f32)
            nc.sync.dma_start(out=cb[:], in_=cblob)
            iota_u = singles.tile([P, 1], u32)
            nc.sync.dma_start(out=iota_u[:], in_=iotad)
            b1_sb = cb[:, 0:KT]                       # b1_sb[p,m] = b1[m*128+p]
            w2col = lambda j: cb[:, KT + j * E:KT + (j + 1) * E]  # W2[j*128+p, e]
            ident = cb[0:E, KT + KT * E:KT + KT * E + E]          # eye(8)
            b2_sb = cb[0:E, CBLOB - 1:CBLOB]

            xh_s, xl_s, w1h_q, w1l_q = [], [], [], []
            for s in range(2):  # seg0, seg1 x before w1 (k-outer needs x first)
                th = singles.tile([P, KT, SW], f16, tag="xhs", name=f"xhs{s}",
                                  bufs=NSEG)
                nc.sync.dma_start(out=th[:], in_=xh[s])
                tl = singles.tile([P, KT, SW], bf16, tag="xls", name=f"xls{s}",
                                  bufs=NSEG)
                nc.sync.dma_start(out=tl[:], in_=xl[s])
                xh_s.append(th)
                xl_s.append(tl)
            for q in range(NQ):
                th = singles.tile([P, 2, H], f16, tag="w1hq", name=f"w1hq{q}",
                                  bufs=NQ)
                nc.scalar.dma_start(out=th[:], in_=w1h[q])
                tl = singles.tile([P, 2, H], bf16, tag="w1lq", name=f"w1lq{q}",
                                  bufs=NQ)
                nc.scalar.dma_start(out=tl[:], in_=w1l[q])
                w1h_q.append(th)
                w1l_q.append(tl)
            for s in range(2, NSEG):
                th = singles.tile([P, KT, SW], f16, tag="xhs", name=f"xhs{s}",
                                  bufs=NSEG)
                nc.sync.dma_start(out=th[:], in_=xh[s])
                tl = singles.tile([P, KT, SW], bf16, tag="xls", name=f"xls{s}",
                                  bufs=NSEG)
                nc.sync.dma_start(out=tl[:], in_=xl[s])
                xh_s.append(th)
                xl_s.append(tl)

            hT = singles.tile([P, KT, T], f32)  # hT[p,m,t] = relu(x@W1+b1)[t, m*128+p]

            def mm3(ps, si, m, k, start, stop):
                msl = slice(m * P, (m + 1) * P)
                wh = w1h_q[k // 2][:, k % 2, msl]
                wl = w1l_q[k // 2][:, k % 2, msl]
                nc.tensor.matmul(ps[:], lhsT=wh, rhs=xh_s[si][:, k, :],
                                 start=start, stop=False)
                nc.tensor.matmul(ps[:], lhsT=wl, rhs=xh_s[si][:, k, :],
                                 start=False, stop=False)
                nc.tensor.matmul(ps[:], lhsT=wh, rhs=xl_s[si][:, k, :],
                                 start=False, stop=stop)

            def relu_stage3(ps, ps3, m, sl):
                nc.scalar.activation(
                    out=hT[:, m, sl], in_=ps[:],
                    func=mybir.ActivationFunctionType.Relu,
                    bias=b1_sb[:, m:m + 1], scale=1.0)
                nc.tensor.matmul(
                    ps3[:], lhsT=w2col(m), rhs=hT[:, m, sl],
                    start=(m == 0), stop=(m == KT - 1))

            # phase A: softmax, top-2, gather launch for one 128-token chunk
            def chunk_phase_a(lT, c0, tch):
                a = tch - c0
                pl = psum.tile([P, E], f32, tag="ps", name="pl",
                               padded_shape=[P, HAL])
                nc.tensor.transpose(pl[:], lT[:, a * P:(a + 1) * P], ident)
                negmax = smalls.tile([P, 1], f32, tag="negmax", name="negmax")
                nc.vector.reduce_max(negmax[:], pl[:],
                                     axis=mybir.AxisListType.X, negate=True)
                exps = smalls.tile([P, E], f32, tag="exps", name="exps")
                nc.scalar.activation(exps[:], pl[:],
                                     func=mybir.ActivationFunctionType.Exp,
                                     bias=negmax[:], scale=1.0)
                ssum = smalls.tile([P, 1], f32, tag="ssum", name="ssum")
                nc.vector.reduce_sum(ssum[:], exps[:], axis=mybir.AxisListType.X)
                rs = smalls.tile([P, 1], f32, tag="rs", name="rs")
                nc.vector.reciprocal(rs[:], ssum[:])
                # top-2 of exps == top-2 of probs; gate = exp * (1/sum)
                mx8 = smalls.tile([P, 8], f32, tag="mx8", name="mx8")
                nc.vector.max(mx8[:], exps[:])
                idx8 = smalls.tile([P, 8], u32, tag="idx8", name="idx8")
                nc.vector.max_index(idx8[:], mx8[:], exps[:])
                # flat eo row = expert*T + (tch*128 + partition)
                base = smalls.tile([P, 1], u32, tag="base", name="base")
                nc.vector.tensor_scalar_add(base[:], iota_u[:], tch * P)
                rows = smalls.tile([P, 2], u32, tag="rows", name="rows")
                for s in range(2):
                    nc.vector.tensor_scalar(
                        rows[:, s:s + 1], idx8[:, s:s + 1],
                        scalar1=T, scalar2=None, op0=mybir.AluOpType.mult)
                    nc.vector.tensor_tensor(
                        out=rows[:, s:s + 1], in0=rows[:, s:s + 1],
                        in1=base[:], op=mybir.AluOpType.add)
                eo_g = eopool.tile([P, 2, H], f16, tag="eog", name="eog")
                for s in range(2):
                    nc.gpsimd.indirect_dma_start(
                        out=eo_g[:, s, :], out_offset=None, in_=eo,
                        in_offset=bass.IndirectOffsetOnAxis(
                            ap=rows[:, s:s + 1], axis=0))
                g0 = smalls.tile([P, 1], f32, tag="g0", name="g0")
                nc.vector.tensor_mul(g0[:], mx8[:, 0:1], rs[:])
                g1 = smalls.tile([P, 1], f32, tag="g1", name="g1")
                nc.vector.tensor_mul(g1[:], mx8[:, 1:2], rs[:])
                return (tch, eo_g, g0, g1)

            # phase B: weighted combine + output store (deferred).
            # Mid-kernel: both ops on DVE so the scalar relu chain never
            # waits on a gather.  Flush (tail): scalar ACTIVATE + DVE STT
            # so consecutive chunks pipeline across the two engines.
            def chunk_phase_b(st, flush=False):
                tch, eo_g, g0, g1 = st
                acc = accpool.tile([P, H], f32, tag="acc", name="acc")
                if flush:
                    nc.scalar.activation(acc[:], eo_g[:, 0, :],
                                         func=mybir.ActivationFunctionType.Copy,
                                         scale=g0[:])
                else:
                    nc.vector.tensor_scalar_mul(acc[:], eo_g[:, 0, :], g0[:])
                nc.vector.scalar_tensor_tensor(
                    out=acc[:], in0=eo_g[:, 1, :], scalar=g1[:], in1=acc[:],
                    op0=mybir.AluOpType.mult, op1=mybir.AluOpType.add)
                nc.sync.dma_start(out=out[tch * P:(tch + 1) * P, :], in_=acc[:])

            pending = []
            for si, (c0, c1) in enumerate(SEGS):
                sl = slice(c0 * P, c1 * P)
                W = (c1 - c0) * P
                # ---- stage 2 (+ interleaved stage 3 and deferred combines) --
                if si == 0:
                    # k-outer: consume W1/xT k-blocks as the DMA delivers
                    # them.  ps3 must be allocated AFTER the 8 stage-2
                    # accumulators: the pool has exactly 8 slots and a 9th
                    # live tile ahead of them deadlocks the slot ring.
                    ps_m = [psum.tile([P, W], f32, tag="ps", name=f"ps{m}",
                                      padded_shape=[P, HAL]) for m in range(KT)]
                    for k in range(KT):
                        for m in range(KT):
                            mm3(ps_m[m], si, m, k, k == 0, k == KT - 1)
                    ps3 = psum.tile([E, W], f32, tag="ps", name="ps3",
                                    padded_shape=[E, HAL])
                    for m in range(KT):
                        relu_stage3(ps_m[m], ps3, m, sl)
                else:
                    ps3 = psum.tile([E, W], f32, tag="ps", name="ps3",
                                    padded_shape=[E, HAL])
                    for m in range(KT):
                        ps = psum.tile([P, W], f32, tag="ps", name="ps",
                                       padded_shape=[P, HAL])
                        for k in range(KT):
                            mm3(ps, si, m, k, k == 0, k == KT - 1)
                        relu_stage3(ps, ps3, m, sl)
                        # run a deferred combine once its gather is ~2
                        # segments old (keep >=2 chunks in flight)
                        if m in (3, 5, 7) and len(pending) > 2:
                            chunk_phase_b(pending.pop(0))

                lT = ltpool.tile([E, W], f32, tag="lT", name="lT",
                                 padded_shape=[E, HAL])
                nc.scalar.activation(out=lT[:], in_=ps3[:],
                                     func=mybir.ActivationFunctionType.Identity,
                                     bias=b2_sb, scale=1.0)
                # last segment: launch the final chunk's gathers first so
                # the tail's combine chain starts on the critical chunk
                order = range(c0, c1) if si < len(SEGS) - 1 else reversed(range(c0, c1))
                for tch in order:
                    pending.append(chunk_phase_a(lT, c0, tch))
            while pending:
                chunk_phase_b(pending.pop(0), flush=True)

    nc.compile()
    return nc


def _get_nc():
    global _compiled_nc
    if _compiled_nc is None:
        _compiled_nc = _build()
    return _compiled_nc


def _split_hi_lo(a):
    """fp16 hi + bf16 lo split of an fp32 array (lo unscaled; bf16's
    exponent range covers it)."""
    import ml_dtypes
    a = np.asarray(a, dtype=np.float32)
    hi = a.astype(np.float16)
    lo = (a.astype(np.float64) - hi.astype(np.float64)).astype(ml_dtypes.bfloat16)
    return hi, lo


def make_in_maps(hidden_states, expert_outputs, W1, b1, W2, b2):
    hs = np.ascontiguousarray(np.asarray(hidden_states, dtype=np.float32)).reshape(B * S, H)
    eo = np.asarray(expert_outputs, dtype=np.float32).reshape(E, B * S, H)
    w1hi, w1lo = _split_hi_lo(W1)
    # w1 k-quarter blobs in SBUF layout: [q, p, kk, m] = W1[(2q+kk)*128+p, m]
    w1hq = np.ascontiguousarray(
        w1hi.reshape(NQ, 2, P, H).transpose(0, 2, 1, 3))
    w1lq = np.ascontiguousarray(
        w1lo.reshape(NQ, 2, P, H).transpose(0, 2, 1, 3))
    b1v = np.asarray(b1, dtype=np.float32)
    w2 = np.asarray(W2, dtype=np.float32)
    b2v = np.asarray(b2, dtype=np.float32)
    # constants blob: b1 | w2 | ident | b2, all in on-chip layout
    cb = np.zeros((P, CBLOB), dtype=np.float32)
    cb[:, 0:KT] = b1v.reshape(KT, P).T                    # b1[m*128+p]
    cb[:, KT:KT + KT * E] = w2.reshape(KT, P, E).transpose(1, 0, 2).reshape(P, KT * E)
    cb[0:E, KT + KT * E:KT + KT * E + E] = np.eye(E, dtype=np.float32)
    cb[0:E, CBLOB - 1] = b2v
    iotav = np.arange(P, dtype=np.uint32).reshape(P, 1)
    in_maps = []
    for c in range(N_CORES):
        sl = slice(c * T, (c + 1) * T)
        xhi, xlo = _split_hi_lo(hs[sl].T)  # [H, T]
        # per-seg SBUF-layout blobs: [s, p, k, u] = xT[k*128+p, s*SW+u]
        xhb = np.ascontiguousarray(
            xhi.reshape(KT, P, NSEG, SW).transpose(2, 1, 0, 3))
        xlb = np.ascontiguousarray(
            xlo.reshape(KT, P, NSEG, SW).transpose(2, 1, 0, 3))
        in_maps.append({
            "xh": xhb, "xl": xlb,
            "eo": np.ascontiguousarray(
                eo[:, sl, :].reshape(E * T, H).astype(np.float16)),
            "w1h": w1hq, "w1l": w1lq, "cblob": cb, "iota": iotav,
        })
    return in_maps


def kernel(hidden_states, expert_outputs, W1, b1, W2, b2, k=2):
    from concourse.bass_utils import run_bass_kernel_spmd

    in_maps = make_in_maps(hidden_states, expert_outputs, W1, b1, W2, b2)
    nc = _get_nc()
    res = run_bass_kernel_spmd(nc, in_maps, core_ids=list(range(N_CORES)))
    full = np.concatenate([res.results[c]["out"] for c in range(N_CORES)], axis=0)
    return full.reshape(B, S, H)


# revision 21
# speedup vs baseline: 1.0252x; 1.0112x over previous
"""MoE expert-gating kernel for 8 Trainium2 NeuronCores.

Problem (nn_ExpertGating): router MLP (H->H relu, H->E) + softmax + top-2
gating + weighted combine of per-expert outputs.

Sharding: data-parallel over the B*S=8192 tokens -> 1024 tokens per core.
Each core runs the full router for its tokens and combines its slice of all
8 experts' outputs.  No collectives needed; host concatenates the slices.

Per-core pipeline (T=1024 tokens, H=1024, E=8):
  1. x arrives host-pre-transposed, pre-split into fp16 hi + bf16 lo
     halves, AND pre-shuffled into the exact SBUF layout per 256-token
     segment, so every input DMA moves 4KB-contiguous partition lines
     (512B-run rearranges measured ~60GB/s vs ~250GB/s for 4KB runs).
  2. hT = relu(W1.T @ xT + b1) via 3 fp16/bf16 matmul passes (hi*hi;
     hi*lo + lo*hi), 1 cycle/row each.  This is the PE floor: the PE
     multiplies FP22 (~12-13 bit) operands, and both x and W1 need ~19
     bits for the logits to rank top-2 correctly (min top-2/3 margin on
     this data is ~5e-6; fp16x3 keeps logit error ~1e-6).  fp32 matmul is
     4 cycles/row (2 HW instructions) and f32r truncates both sides to
     ~12 bits (probed on HW), so neither beats 3x fp16.
  3. logitsT[e, t] += W2.T @ hT accumulated per m-block right after each
     relu (fp32: W2 also needs >13 bits), so segment logits are ready
     immediately after the segment's last stage-2 matmul.
  4. transpose logit chunks to [t, E] via PE (8x8 identity), softmax,
     top-2 via max8 + max_index, and launch the indirect-DMA gathers of
     each token's 2 selected expert rows.  expert_outputs are host-cast
     to fp16 (4 MB gathered instead of 8; rounding error ~2e-4 rel vs
     the 2e-2 budget): the SWDGE gather queue runs at only ~90-180GB/s
     and is the tail bottleneck.
  5. the weighted combine (out = g0*row0 + g1*row1) for segment s's
     chunks is DEFERRED TWO segments (emitted mid segment s+2's m-loop)
     and runs on the DVE: engine queues are strictly in-order, and a
     combine emitted earlier waits on its gather transfer,
     head-of-line-blocking the relu chain and stalling the PE.

DMA notes: every dma_start costs ~610ns of dispatch on the issuing queue,
so inputs are batched into a few transfers split across BOTH HWDGE rings
(SP: constants + xT segments; Activation: W1 k-quarters), each a separate
tile so tile-level dependencies stay fine-grained.  Segment 0 runs its
k-loop outermost, consuming W1 k-quarters as they land.
"""

import numpy as np

B, S, H, E = 4, 2048, 1024, 8
N_CORES = 8
T = (B * S) // N_CORES  # tokens per core
P = 128  # partitions
TCH = T // P  # token chunks per core (8)
KT = H // P  # contraction tiles (8)
HAL = 512  # psum pad width (full bank)
SEGS = [(0, 2), (2, 4), (4, 6), (6, 8)]
NSEG = len(SEGS)
SW = 2 * P  # segment width (tokens)
NQ = KT // 2  # w1 delivered in k-quarters
CBLOB = KT + KT * E + E + 1  # b1 | w2 | ident | b2  = 81 fp32 columns

_compiled_nc = None


def _build():
    import concourse.bacc as bacc
    import concourse.bass as bass
    import concourse.tile as tile
    from concourse import mybir

    f32 = mybir.dt.float32
    f16 = mybir.dt.float16
    bf16 = mybir.dt.bfloat16
    u32 = mybir.dt.uint32
    nc = bacc.Bacc("TRN2", target_bir_lowering=False, debug=False,
                   num_devices=N_CORES)

    # all inputs are host-shuffled to exact SBUF layout (4KB partition lines)
    xh = nc.dram_tensor("xh", [NSEG, P, KT, SW], f16, kind="ExternalInput").ap()
    xl = nc.dram_tensor("xl", [NSEG, P, KT, SW], bf16, kind="ExternalInput").ap()
    eo = nc.dram_tensor("eo", [E * T, H], f16, kind="ExternalInput").ap()
    w1h = nc.dram_tensor("w1h", [NQ, P, 2, H], f16, kind="ExternalInput").ap()
    w1l = nc.dram_tensor("w1l", [NQ, P, 2, H], bf16, kind="ExternalInput").ap()
    cblob = nc.dram_tensor("cblob", [P, CBLOB], f32, kind="ExternalInput").ap()
    iotad = nc.dram_tensor("iota", [P, 1], u32, kind="ExternalInput").ap()
    out = nc.dram_tensor("out", [T, H], f32, kind="ExternalOutput").ap()

    with tile.TileContext(nc) as tc:
        with (
            tc.tile_pool(name="singles", bufs=1) as singles,
            tc.tile_pool(name="eopool", bufs=6) as eopool,
            tc.tile_pool(name="accpool", bufs=3) as accpool,
            tc.tile_pool(name="smalls", bufs=8) as smalls,
            tc.tile_pool(name="ltpool", bufs=2) as ltpool,
            tc.tile_pool(name="psum", bufs=8, space="PSUM") as psum,
        ):
            # ---- input DMAs: SP ring = constants + xT, Activation ring =
            # W1 k-quarters (separate tiles per transfer -> fine deps) ----
            cb = singles.tile([P, CBLOB], f32)
            nc.sync.dma_start(out=cb[:], in_=cblob)
            iota_u = singles.tile([P, 1], u32)
            nc.sync.dma_start(out=iota_u[:], in_=iotad)
            b1_sb = cb[:, 0:KT]                       # b1_sb[p,m] = b1[m*128+p]
            w2col = lambda j: cb[:, KT + j * E:KT + (j + 1) * E]  # W2[j*128+p, e]
            ident = cb[0:E, KT + KT * E:KT + KT * E + E]          # eye(8)
            b2_sb = cb[0:E, CBLOB - 1:CBLOB]

            xh_s, xl_s, w1h_q, w1l_q = [], [], [], []
            for s in range(2):  # seg0, seg1 x before w1 (k-outer needs x first)
                th = singles.tile([P, KT, SW], f16, tag="xhs", name=f"xhs{s}",
                                  bufs=NSEG)
                nc.sync.dma_start(out=th[:], in_=xh[s])
                tl = singles.tile([P, KT, SW], bf16, tag="xls", name=f"xls{s}",
                                  bufs=NSEG)
                nc.sync.dma_start(out=tl[:], in_=xl[s])
                xh_s.append(th)
                xl_s.append(tl)
            for q in range(NQ):
                th = singles.tile([P, 2, H], f16, tag="w1hq", name=f"w1hq{q}",
                                  bufs=NQ)
                nc.scalar.dma_start(out=th[:], in_=w1h[q])
                tl = singles.tile([P, 2, H], bf16, tag="w1lq", name=f"w1lq{q}",
                                  bufs=NQ)
                nc.scalar.dma_start(out=tl[:], in_=w1l[q])
                w1h_q.append(th)
                w1l_q.append(tl)
            for s in range(2, NSEG):
                th = singles.tile([P, KT, SW], f16, tag="xhs", name=f"xhs{s}",
                                  bufs=NSEG)
                nc.sync.dma_start(out=th[:], in_=xh[s])
                tl = singles.tile([P, KT, SW], bf16, tag="xls", name=f"xls{s}",
                                  bufs=NSEG)
                nc.sync.dma_start(out=tl[:], in_=xl[s])
                xh_s.append(th)
                xl_s.append(tl)

            hT = singles.tile([P, KT, T], f32)  # hT[p,m,t] = relu(x@W1+b1)[t, m*128+p]

            def mm3(ps, si, m, k, start, stop):
                msl = slice(m * P, (m + 1) * P)
                wh = w1h_q[k // 2][:, k % 2, msl]
                wl = w1l_q[k // 2][:, k % 2, msl]
                nc.tensor.matmul(ps[:], lhsT=wh, rhs=xh_s[si][:, k, :],
                                 start=start, stop=False)
                nc.tensor.matmul(ps[:], lhsT=wl, rhs=xh_s[si][:, k, :],
                                 start=False, stop=False)
                nc.tensor.matmul(ps[:], lhsT=wh, rhs=xl_s[si][:, k, :],
                                 start=False, stop=stop)

            def relu_stage3(ps, ps3, m, sl):
                nc.scalar.activation(
                    out=hT[:, m, sl], in_=ps[:],
                    func=mybir.ActivationFunctionType.Relu,
                    bias=b1_sb[:, m:m + 1], scale=1.0)
                nc.tensor.matmul(
                    ps3[:], lhsT=w2col(m), rhs=hT[:, m, sl],
                    start=(m == 0), stop=(m == KT - 1))

            # phase A: softmax, top-2, gather launch for one 128-token chunk
            def chunk_phase_a(lT, c0, tch):
                a = tch - c0
                pl = psum.tile([P, E], f32, tag="ps", name="pl",
                               padded_shape=[P, HAL])
                nc.tensor.transpose(pl[:], lT[:, a * P:(a + 1) * P], ident)
                negmax = smalls.tile([P, 1], f32, tag="negmax", name="negmax")
                nc.vector.reduce_max(negmax[:], pl[:],
                                     axis=mybir.AxisListType.X, negate=True)
                exps = smalls.tile([P, E], f32, tag="exps", name="exps")
                nc.scalar.activation(exps[:], pl[:],
                                     func=mybir.ActivationFunctionType.Exp,
                                     bias=negmax[:], scale=1.0)
                ssum = smalls.tile([P, 1], f32, tag="ssum", name="ssum")
                nc.vector.reduce_sum(ssum[:], exps[:], axis=mybir.AxisListType.X)
                rs = smalls.tile([P, 1], f32, tag="rs", name="rs")
                nc.vector.reciprocal(rs[:], ssum[:])
                # top-2 of exps == top-2 of probs; gate = exp * (1/sum)
                mx8 = smalls.tile([P, 8], f32, tag="mx8", name="mx8")
                nc.vector.max(mx8[:], exps[:])
                idx8 = smalls.tile([P, 8], u32, tag="idx8", name="idx8")
                nc.vector.max_index(idx8[:], mx8[:], exps[:])
                # flat eo row = expert*T + (tch*128 + partition)
                base = smalls.tile([P, 1], u32, tag="base", name="base")
                nc.vector.tensor_scalar_add(base[:], iota_u[:], tch * P)
                rows = smalls.tile([P, 2], u32, tag="rows", name="rows")
                for s in range(2):
                    nc.vector.tensor_scalar(
                        rows[:, s:s + 1], idx8[:, s:s + 1],
                        scalar1=T, scalar2=None, op0=mybir.AluOpType.mult)
                    nc.vector.tensor_tensor(
                        out=rows[:, s:s + 1], in0=rows[:, s:s + 1],
                        in1=base[:], op=mybir.AluOpType.add)
                eo_g = eopool.tile([P, 2, H], f16, tag="eog", name="eog")
                # NOTE: a single [P,2]-offset indirect DMA compiles and
                # simulates but dies at runtime (NRT INTERNAL) -- keep two
                # single-offset gathers
                for s in range(2):
                    nc.gpsimd.indirect_dma_start(
                        out=eo_g[:, s, :], out_offset=None, in_=eo,
                        in_offset=bass.IndirectOffsetOnAxis(
                            ap=rows[:, s:s + 1], axis=0))
                g0 = smalls.tile([P, 1], f32, tag="g0", name="g0")
                nc.vector.tensor_mul(g0[:], mx8[:, 0:1], rs[:])
                g1 = smalls.tile([P, 1], f32, tag="g1", name="g1")
                nc.vector.tensor_mul(g1[:], mx8[:, 1:2], rs[:])
                return (tch, eo_g, g0, g1)

            # phase B: weighted combine + output store (deferred).
            # Mid-kernel: both ops on DVE so the scalar relu chain never
            # waits on a gather.  Flush (tail): scalar ACTIVATE + DVE STT
            # so consecutive chunks pipeline across the two engines.
            def chunk_phase_b(st, flush=False):
                tch, eo_g, g0, g1 = st
                acc = accpool.tile([P, H], f32, tag="acc", name="acc")
                if flush:
                    nc.scalar.activation(acc[:], eo_g[:, 0, :],
                                         func=mybir.ActivationFunctionType.Copy,
                                         scale=g0[:])
                else:
                    nc.vector.tensor_scalar_mul(acc[:], eo_g[:, 0, :], g0[:])
                nc.vector.scalar_tensor_tensor(
                    out=acc[:], in0=eo_g[:, 1, :], scalar=g1[:], in1=acc[:],
                    op0=mybir.AluOpType.mult, op1=mybir.AluOpType.add)
                nc.sync.dma_start(out=out[tch * P:(tch + 1) * P, :], in_=acc[:])

            pending = []
            for si, (c0, c1) in enumerate(SEGS):
                sl = slice(c0 * P, c1 * P)
                W = (c1 - c0) * P
                # ---- stage 2 (+ interleaved stage 3 and deferred combines) --
                if si == 0:
                    # k-outer: consume W1/xT k-blocks as the DMA delivers
                    # them.  ps3 must be allocated AFTER the 8 stage-2
                    # accumulators: the pool has exactly 8 slots and a 9th
                    # live tile ahead of them deadlocks the slot ring.
                    ps_m = [psum.tile([P, W], f32, tag="ps", name=f"ps{m}",
                                      padded_shape=[P, HAL]) for m in range(KT)]
                    for k in range(KT):
                        for m in range(KT):
                            mm3(ps_m[m], si, m, k, k == 0, k == KT - 1)
                    ps3 = psum.tile([E, W], f32, tag="ps", name="ps3",
                                    padded_shape=[E, HAL])
                    for m in range(KT):
                        relu_stage3(ps_m[m], ps3, m, sl)
                else:
                    ps3 = psum.tile([E, W], f32, tag="ps", name="ps3",
                                    padded_shape=[E, HAL])
                    for m in range(KT):
                        ps = psum.tile([P, W], f32, tag="ps", name="ps",
                                       padded_shape=[P, HAL])
                        for k in range(KT):
                            mm3(ps, si, m, k, k == 0, k == KT - 1)
                        relu_stage3(ps, ps3, m, sl)
                        # run a deferred combine once its gather is ~2
                        # segments old (keep >=2 chunks in flight)
                        if m in (3, 5, 7) and len(pending) > 2:
                            chunk_phase_b(pending.pop(0))

                lT = ltpool.tile([E, W], f32, tag="lT", name="lT",
                                 padded_shape=[E, HAL])
                nc.scalar.activation(out=lT[:], in_=ps3[:],
                                     func=mybir.ActivationFunctionType.Identity,
                                     bias=b2_sb, scale=1.0)
                # last segment: launch the final chunk's gathers first so
                # the tail's combine chain starts on the critical chunk
                order = range(c0, c1) if si < len(SEGS) - 1 else reversed(range(c0, c1))
                for tch in order:
                    pending.append(chunk_phase_a(lT, c0, tch))
            while pending:
                chunk_phase_b(pending.pop(0), flush=True)

    nc.compile()
    return nc


def _get_nc():
    global _compiled_nc
    if _compiled_nc is None:
        _compiled_nc = _build()
    return _compiled_nc


def _split_hi_lo(a):
    """fp16 hi + bf16 lo split of an fp32 array (lo unscaled; bf16's
    exponent range covers it)."""
    import ml_dtypes
    a = np.asarray(a, dtype=np.float32)
    hi = a.astype(np.float16)
    lo = (a.astype(np.float64) - hi.astype(np.float64)).astype(ml_dtypes.bfloat16)
    return hi, lo


def make_in_maps(hidden_states, expert_outputs, W1, b1, W2, b2):
    hs = np.ascontiguousarray(np.asarray(hidden_states, dtype=np.float32)).reshape(B * S, H)
    eo = np.asarray(expert_outputs, dtype=np.float32).reshape(E, B * S, H)
    w1hi, w1lo = _split_hi_lo(W1)
    # w1 k-quarter blobs in SBUF layout: [q, p, kk, m] = W1[(2q+kk)*128+p, m]
    w1hq = np.ascontiguousarray(
        w1hi.reshape(NQ, 2, P, H).transpose(0, 2, 1, 3))
    w1lq = np.ascontiguousarray(
        w1lo.reshape(NQ, 2, P, H).transpose(0, 2, 1, 3))
    b1v = np.asarray(b1, dtype=np.float32)
    w2 = np.asarray(W2, dtype=np.float32)
    b2v = np.asarray(b2, dtype=np.float32)
    # constants blob: b1 | w2 | ident | b2, all in on-chip layout
    cb = np.zeros((P, CBLOB), dtype=np.float32)
    cb[:, 0:KT] = b1v.reshape(KT, P).T                    # b1[m*128+p]
    cb[:, KT:KT + KT * E] = w2.reshape(KT, P, E).transpose(1, 0, 2).reshape(P, KT * E)
    cb[0:E, KT + KT * E:KT + KT * E + E] = np.eye(E, dtype=np.float32)
    cb[0:E, CBLOB - 1] = b2v
    iotav = np.arange(P, dtype=np.uint32).reshape(P, 1)
    in_maps = []
    for c in range(N_CORES):
        sl = slice(c * T, (c + 1) * T)
        xhi, xlo = _split_hi_lo(hs[sl].T)  # [H, T]
        # per-seg SBUF-layout blobs: [s, p, k, u] = xT[k*128+p, s*SW+u]
        xhb = np.ascontiguousarray(
            xhi.reshape(KT, P, NSEG, SW).transpose(2, 1, 0, 3))
        xlb = np.ascontiguousarray(
            xlo.reshape(KT, P, NSEG, SW).transpose(2, 1, 0, 3))
        in_maps.append({
            "xh": xhb, "xl": xlb,
            "eo": np.ascontiguousarray(
                eo[:, sl, :].reshape(E * T, H).astype(np.float16)),
            "w1h": w1hq, "w1l": w1lq, "cblob": cb, "iota": iotav,
        })
    return in_maps


def kernel(hidden_states, expert_outputs, W1, b1, W2, b2, k=2):
    from concourse.bass_utils import run_bass_kernel_spmd

    in_maps = make_in_maps(hidden_states, expert_outputs, W1, b1, W2, b2)
    nc = _get_nc()
    res = run_bass_kernel_spmd(nc, in_maps, core_ids=list(range(N_CORES)))
    full = np.concatenate([res.results[c]["out"] for c in range(N_CORES)], axis=0)
    return full.reshape(B, S, H)
